# revision 12
# baseline (speedup 1.0000x reference)
"""Trainium2 Bass kernel for nn_Block_9397388444369.

Reference semantics (B=2, T=512, C=256, HID=1024):
    transform = (h @ Wt.T) * 0.0  -> attention branch is exactly bp
    x1  = x + bp
    ff  = relu(LN(x1,g2,b2) @ W1.T + bf1) @ W2.T + bf2
    out = x1 + ff

Device computes only the MLP partials; x1/bp/bf2 are added on the host in
fp32 (exact). LayerNorm is folded into the matmuls:

  z[m,t] = rstd[t] * (sum_c x1[t,c] w1t[c,m] - mu[t] s1[m] + sigma[t] bf1[m])

The "-mu s1 + sigma bf1" term rides as a 2-row augmented matmul (lhsT =
[-s1; bf1] delivered as per-partition columns and PE-transposed on device,
rhs = [mu; sigma] from PE-transposing the bn_stats output), so mm1 runs on
RAW host-transposed x and only the last accumulation waits on stats.
rstd > 0 commutes through the ReLU and is applied once at mm2's fp32
output (t = partition dim there).

mm1/mm2 run in fp8 (e4m3, TRN max +-240) with power-of-2 weight scales
S1=S2=1024 and a 1/16 relu rescale, folded into the final per-partition
multiply (rstd/65536) -- exact in binary. DoubleRow perf mode contracts
both k-chunks in one matmul (2 MACs/cell/cycle). Stats read the fp8 row
copy (simulated end-to-end error ~7.5e-3 vs the 2e-2 gate).

DMA: the stats rows (64KB) + aux (34KB) ride the Sync queue; the big
weights+xt blob (320KB, 2.5KB per-partition lines) rides the Scalar queue
concurrently -- per-partition lines >=2KB are what keep HBM near peak.
ReLUs run as [128,128] halves alternating Scalar/Vector so mm2's first
row-tile starts as soon as its half is ready. A dozen dummy matmuls on a
memset tile keep the PE busy from kernel start so the HAM clock gate
lifts (1.2->2.4 GHz) before the real matmuls arrive; the stats/transpose
path is traced under tc.high_priority() so the static schedule prefers
it the moment its inputs land.
"""

import sys

if '/opt/trn_rl_repo' not in sys.path:
    sys.path.insert(0, '/opt/trn_rl_repo')

import ml_dtypes
import numpy as np

import concourse.bass as bass  # noqa: F401
import concourse.tile as tile
from concourse import bacc, mybir
from concourse.bass_utils import run_bass_kernel_spmd

B, T, C = 2, 512, 256
HID = 4 * C
EPS = 1e-5
N_CORES = 8
N_GROUPS = 4                       # row groups
ROWS = (B * T) // N_GROUPS         # 256 rows per core
RT = ROWS // 128                   # 2 row tiles per core
HH = HID // 2                      # 512-wide hidden half per core
KC = C // 128                      # 2 k-subtiles over C
KH = HH // 128                     # 4 m-chunks over the half
N_WARM = 12                        # dummy matmuls to lift the HAM clock gate

F32 = mybir.dt.float32
BF16 = mybir.dt.bfloat16
FP8 = mybir.dt.float8e4
BF16_NP = ml_dtypes.bfloat16
FP8_NP = ml_dtypes.float8_e4m3

S1 = 1024.0                        # w1 scale (power of 2)
S2 = 1024.0                        # w2 scale
SR = 1.0 / 16.0                    # relu output rescale
STOT = S1 * S2 * SR                # folded into the final rstd multiply

# big8 plane layout (per k-plane): [xt_k (256) | w1t_k (512) | w2_plane (512)]
PW1 = 256                          # w1t offset within a plane
PW2 = 768                          # w2 offset within a plane
PCOLS = 1280


def _build_nc():
    nc = bacc.Bacc("TRN2", target_bir_lowering=False, debug=False,
                   num_devices=N_CORES)

    xr8_d = nc.declare_dram_parameter("xr8", [128, RT * C], FP8,
                                      isOutput=False)
    aux_d = nc.declare_dram_parameter("aux", [128, 128 + 2 * KH], BF16,
                                      isOutput=False)
    big8_d = nc.declare_dram_parameter("big8", [128, KC, PCOLS], FP8,
                                       isOutput=False)
    y_d = nc.declare_dram_parameter("y", [128, RT * C], BF16, isOutput=True)

    DR = mybir.MatmulPerfMode.DoubleRow

    with tile.TileContext(nc) as tc:
        with (
            tc.tile_pool(name="acts", bufs=1) as acts,
            tc.tile_pool(name="stats", bufs=2) as stats,
            tc.tile_pool(name="ptrans", bufs=2, space="PSUM") as ptrans,
            tc.tile_pool(name="pmm1", bufs=4, space="PSUM") as pmm1,
            tc.tile_pool(name="pmm2", bufs=2, space="PSUM") as pmm2,
        ):
            # ---- input DMAs: small early blobs on Sync, big blob on Scalar
            xr8_sb = acts.tile([128, RT * C], FP8)
            nc.sync.dma_start(out=xr8_sb, in_=xr8_d.ap())
            aux_sb = acts.tile([128, 128 + 2 * KH], BF16)
            nc.sync.dma_start(out=aux_sb, in_=aux_d.ap())
            big8_sb = acts.tile([128, KC, PCOLS], FP8)
            nc.scalar.dma_start(out=big8_sb, in_=big8_d.ap())

            eps_t = acts.tile([128, 1], F32)
            nc.vector.memset(eps_t, np.float32(EPS))
            warm_src = acts.tile([128, 256], BF16)
            nc.vector.memset(warm_src, np.float32(0.5))

            ident = aux_sb[:, 0:128]

            with tc.high_priority():
                # ---- stats per row tile ----
                # aug_rhs[0,t] = mu[t]; aug_rhs[1,t] = sqrt(var[t]+eps)
                aug_rhs = acts.tile([2, ROWS], BF16)
                rstd_s = []
                stgs = []
                for r in range(RT):
                    xr = xr8_sb[:, r * C:(r + 1) * C]
                    bn6 = stats.tile([128, 6], F32, tag="bn6")
                    nc.vector.bn_stats(out=bn6, in_=xr)
                    mv = stats.tile([128, 2], F32, tag="mv")
                    nc.vector.bn_aggr(out=mv, in_=bn6)
                    stg = stats.tile([128, 2], BF16, tag="stg")
                    nc.scalar.activation(
                        out=stg[:, 1:2], in_=mv[:, 1:2],
                        func=mybir.ActivationFunctionType.Sqrt,
                        bias=eps_t, scale=1.0)
                    nc.vector.tensor_copy(out=stg[:, 0:1], in_=mv[:, 0:1])
                    stgs.append(stg)
                    # rstd/STOT for the final scale (off the aug path)
                    rstd = stats.tile([128, 1], F32, tag="rstd")
                    nc.vector.reciprocal(out=rstd, in_=stg[:, 1:2])
                    rs = stats.tile([128, 1], F32, tag="rs")
                    nc.vector.tensor_scalar_mul(rs, rstd, 1.0 / STOT)
                    rstd_s.append(rs)

                # ---- stat transposes, then augw assembly (PE idle window)
                for r in range(RT):
                    pt = ptrans.tile([2, 128], BF16, tag="pt", name=f"pt_{r}")
                    nc.tensor.transpose(pt, stgs[r], ident)
                    nc.vector.tensor_copy(
                        out=aug_rhs[:, r * 128:(r + 1) * 128], in_=pt)

                augw_sb = acts.tile([2, HH], BF16)
                for mc in range(KH):
                    pa = ptrans.tile([2, 128], BF16, tag="pt",
                                     name=f"pa_{mc}")
                    nc.tensor.transpose(
                        pa, aux_sb[:, 128 + 2 * mc:130 + 2 * mc], ident)
                    nc.vector.tensor_copy(
                        out=augw_sb[:, mc * 128:(mc + 1) * 128], in_=pa)

            # ---- raw mm1 (fp8 DR: both k-chunks in one matmul) ----
            ps1 = [pmm1.tile([128, ROWS], F32, tag=f"ps1_{i}", bufs=1,
                             name=f"ps1_{i}")
                   for i in range(KH)]
            for mc in range(KH):
                nc.tensor.matmul(
                    ps1[mc],
                    lhsT=big8_sb[:, :, PW1 + mc * 128:PW1 + (mc + 1) * 128],
                    rhs=big8_sb[:, :, 0:256],
                    start=True, stop=False,
                    perf_mode=DR,
                )

            # ---- aug matmul (bf16), then relu halves on both engines ----
            relu1 = acts.tile([128, KH, ROWS], FP8)
            for mc in range(KH):
                nc.tensor.matmul(
                    ps1[mc],
                    lhsT=augw_sb[:, mc * 128:(mc + 1) * 128],
                    rhs=aug_rhs,
                    start=False, stop=True,
                )
            for r in range(RT):
                for mc in range(KH):
                    sl = slice(r * 128, (r + 1) * 128)
                    if (mc + r) % 2 == 0:
                        nc.scalar.activation(
                            out=relu1[:, mc, sl], in_=ps1[mc][:, sl],
                            func=mybir.ActivationFunctionType.Relu,
                            bias=0.0, scale=float(SR))
                    else:
                        nc.vector.tensor_scalar(
                            out=relu1[:, mc, sl], in0=ps1[mc][:, sl],
                            scalar1=0.0, scalar2=float(SR),
                            op0=mybir.AluOpType.max,
                            op1=mybir.AluOpType.mult)

            # ---- mm2 (fp8 DR) + final rstd/STOT scale per row tile ----
            y_sb = acts.tile([128, RT, C], BF16)
            for r in range(RT):
                po = pmm2.tile([128, C], F32)
                for j in range(KH // 2):
                    nc.tensor.matmul(
                        po,
                        lhsT=relu1[:, 2 * j:2 * j + 2, r * 128:(r + 1) * 128],
                        rhs=big8_sb[:, :, PW2 + j * C:PW2 + (j + 1) * C],
                        start=(j == 0), stop=(j == KH // 2 - 1),
                        perf_mode=DR,
                    )
                if r == 0:
                    nc.vector.tensor_scalar_mul(y_sb[:, 0, :], po, rstd_s[0])
                    nc.sync.dma_start(out=y_d.ap()[:, :C], in_=y_sb[:, 0, :])
                else:
                    nc.scalar.activation(
                        out=y_sb[:, 1, :], in_=po,
                        func=mybir.ActivationFunctionType.Copy,
                        bias=0.0, scale=rstd_s[1])
                    nc.scalar.dma_start(out=y_d.ap()[:, C:],
                                        in_=y_sb[:, 1, :])

            # ---- PE warm-up fillers: no deps, lowest priority, the
            # scheduler slots them into PE idle gaps from kernel start ----
            for i in range(N_WARM):
                pw = ptrans.tile([128, 256], F32, tag="pt", name=f"pw_{i}")
                nc.tensor.matmul(pw, lhsT=warm_src[:, 0:128], rhs=warm_src,
                                 start=True, stop=True)

    nc.finalize()
    return nc


_NC_CACHE = None


def _get_nc():
    global _NC_CACHE
    if _NC_CACHE is None:
        _NC_CACHE = _build_nc()
    return _NC_CACHE


def _q8(a, scale):
    s = np.asarray(a, dtype=np.float64) * scale
    s = np.clip(s, -240.0, 240.0)
    return s.astype(np.float32).astype(FP8_NP)


def _pack_inputs(x, bp, g2, b2, W1, bf1, W2):
    x1 = (np.asarray(x, dtype=np.float64).reshape(B * T, C)
          + np.asarray(bp, dtype=np.float64))
    x1_f32 = x1.astype(np.float32)
    x1_f8 = x1_f32.astype(BF16_NP).astype(np.float32).astype(FP8_NP)

    w1t_eff = (np.asarray(W1).astype(np.float64).T
               * np.asarray(g2).astype(np.float64)[:, None])      # [C, HID]
    w1t_f8 = _q8(w1t_eff, S1)
    bf1_eff = (np.asarray(bf1).astype(np.float64)
               + np.asarray(b2).astype(np.float64)
               @ np.asarray(W1).astype(np.float64).T)             # [HID]
    # aug row0 = -sum_c of the scaled fp8 weights actually used
    s1_scaled = w1t_f8.astype(np.float64).sum(axis=0)             # S1-scaled
    aug0 = (-s1_scaled).astype(np.float32).astype(BF16_NP)
    aug1 = (bf1_eff * S1).astype(np.float32).astype(BF16_NP)
    w2t_f8 = _q8(np.asarray(W2, dtype=np.float64).T, S2)          # [HID, C]
    ident = np.eye(128, dtype=np.float32).astype(BF16_NP)

    in_maps = []
    for c in range(N_CORES):
        g, hf = c // 2, c % 2
        xg_f8 = x1_f8[g * ROWS:(g + 1) * ROWS]                    # [256, C]

        xr8 = np.empty((128, RT * C), dtype=FP8_NP)
        for r in range(RT):
            xr8[:, r * C:(r + 1) * C] = xg_f8[r * 128:(r + 1) * 128, :]

        aux = np.empty((128, 128 + 2 * KH), dtype=BF16_NP)
        aux[:, 0:128] = ident
        for mc in range(KH):
            sl = slice(hf * HH + mc * 128, hf * HH + (mc + 1) * 128)
            aux[:, 128 + 2 * mc] = aug0[sl]
            aux[:, 129 + 2 * mc] = aug1[sl]

        big8 = np.empty((128, KC, PCOLS), dtype=FP8_NP)
        w1h = w1t_f8[:, hf * HH:(hf + 1) * HH]                    # [C, HH]
        w2h = w2t_f8[hf * HH:(hf + 1) * HH]                       # [HH, C]
        for k in range(KC):
            big8[:, k, 0:PW1] = xg_f8[:, k * 128:(k + 1) * 128].T
            big8[:, k, PW1:PW2] = w1h[k * 128:(k + 1) * 128, :]
        for j in range(KH // 2):
            for q in range(2):
                mc = 2 * j + q
                big8[:, q, PW2 + j * C:PW2 + (j + 1) * C] = \
                    w2h[mc * 128:(mc + 1) * 128, :]

        in_maps.append({"xr8": xr8, "aux": aux, "big8": big8})
    return in_maps, x1_f32


def _make_in_maps(x, bp, g2, b2, W1, bf1, W2):
    in_maps, _ = _pack_inputs(x, bp, g2, b2, W1, bf1, W2)
    return in_maps


def kernel(x, Wt, Wp, bp, g1, b1, g2, b2, W1, bf1, W2, bf2):
    in_maps, x1_f32 = _pack_inputs(x, bp, g2, b2, W1, bf1, W2)
    nc = _get_nc()
    res = run_bass_kernel_spmd(nc, in_maps, list(range(N_CORES)))

    out = x1_f32.copy()                                       # residual x+bp
    for g in range(N_GROUPS):
        for hf in range(2):
            y = np.asarray(res.results[2 * g + hf]["y"]).astype(np.float32)
            for r in range(RT):
                out[g * ROWS + r * 128:g * ROWS + (r + 1) * 128, :] += \
                    y[:, r * C:(r + 1) * C]
    out = out + np.asarray(bf2, dtype=np.float32)
    return out.reshape(B, T, C).astype(np.float32)


# revision 15
# speedup vs baseline: 1.0033x; 1.0033x over previous
"""Trainium2 Bass kernel for nn_Block_9397388444369.

Reference semantics (B=2, T=512, C=256, HID=1024):
    transform = (h @ Wt.T) * 0.0  -> attention branch is exactly bp
    x1  = x + bp
    ff  = relu(LN(x1,g2,b2) @ W1.T + bf1) @ W2.T + bf2
    out = x1 + ff

Device computes only the MLP partials; x1/bp/bf2 are added on the host in
fp32 (exact). LayerNorm is folded into the matmuls:

  z[m,t] = rstd[t] * (sum_c x1[t,c] w1t[c,m] - mu[t] s1[m] + sigma[t] bf1[m])

The "-mu s1 + sigma bf1" term rides as a 2-row augmented matmul (lhsT =
[-s1; bf1] delivered as per-partition columns and PE-transposed on device,
rhs = [mu; sigma] from PE-transposing the bn_stats output), so mm1 runs on
RAW host-transposed x and only the last accumulation waits on stats.
rstd > 0 commutes through the ReLU and is applied once at mm2's fp32
output (t = partition dim there).

mm1/mm2 run in fp8 (e4m3, TRN max +-240) with power-of-2 weight scales
S1=S2=1024 and a 1/16 relu rescale, folded into the final per-partition
multiply (rstd/65536) -- exact in binary. DoubleRow perf mode contracts
both k-chunks in one matmul (2 MACs/cell/cycle). Stats read the fp8 row
copy (simulated end-to-end error ~7.5e-3 vs the 2e-2 gate).

DMA: the stats rows (64KB) + aux (34KB) ride the Sync queue; the big
weights+xt blob (320KB, 2.5KB per-partition lines) rides the Scalar queue
concurrently -- per-partition lines >=2KB are what keep HBM near peak.
ReLUs run as [128,128] halves alternating Scalar/Vector so mm2's first
row-tile starts as soon as its half is ready. A dozen dummy matmuls on a
memset tile keep the PE busy from kernel start so the HAM clock gate
lifts (1.2->2.4 GHz) before the real matmuls arrive; the stats/transpose
path is traced under tc.high_priority() so the static schedule prefers
it the moment its inputs land.
"""

import sys

if '/opt/trn_rl_repo' not in sys.path:
    sys.path.insert(0, '/opt/trn_rl_repo')

import ml_dtypes
import numpy as np

import concourse.bass as bass  # noqa: F401
import concourse.tile as tile
from concourse import bacc, mybir
from concourse.bass_utils import run_bass_kernel_spmd

B, T, C = 2, 512, 256
HID = 4 * C
EPS = 1e-5
N_CORES = 8
N_GROUPS = 4                       # row groups
ROWS = (B * T) // N_GROUPS         # 256 rows per core
RT = ROWS // 128                   # 2 row tiles per core
HH = HID // 2                      # 512-wide hidden half per core
KC = C // 128                      # 2 k-subtiles over C
KH = HH // 128                     # 4 m-chunks over the half
N_WARM = 12                        # dummy matmuls to lift the HAM clock gate

F32 = mybir.dt.float32
BF16 = mybir.dt.bfloat16
FP8 = mybir.dt.float8e4
BF16_NP = ml_dtypes.bfloat16
FP8_NP = ml_dtypes.float8_e4m3

S1 = 1024.0                        # w1 scale (power of 2)
S2 = 1024.0                        # w2 scale
SR = 1.0 / 16.0                    # relu output rescale
STOT = S1 * S2 * SR                # folded into the final rstd multiply

# big8 plane layout (per k-plane): [xt_k (256) | w1t_k (512) | w2_plane (512)]
PW1 = 256                          # w1t offset within a plane
PW2 = 768                          # w2 offset within a plane
PCOLS = 1280


def _build_nc():
    nc = bacc.Bacc("TRN2", target_bir_lowering=False, debug=False,
                   num_devices=N_CORES)

    xr8_d = nc.declare_dram_parameter("xr8", [128, RT * C], FP8,
                                      isOutput=False)
    aux_d = nc.declare_dram_parameter("aux", [128, 128 + 2 * KH], BF16,
                                      isOutput=False)
    big8_d = nc.declare_dram_parameter("big8", [128, KC, PCOLS], FP8,
                                       isOutput=False)
    y_d = nc.declare_dram_parameter("y", [128, RT * C], BF16, isOutput=True)

    DR = mybir.MatmulPerfMode.DoubleRow

    with tile.TileContext(nc) as tc:
        with (
            tc.tile_pool(name="acts", bufs=1) as acts,
            tc.tile_pool(name="stats", bufs=2) as stats,
            tc.tile_pool(name="ptrans", bufs=2, space="PSUM") as ptrans,
            tc.tile_pool(name="pmm1", bufs=4, space="PSUM") as pmm1,
            tc.tile_pool(name="pmm2", bufs=2, space="PSUM") as pmm2,
        ):
            # ---- input DMAs: small early blobs on Sync, big blob on Scalar
            xr8_sb = acts.tile([128, RT * C], FP8)
            nc.sync.dma_start(out=xr8_sb, in_=xr8_d.ap())
            aux_sb = acts.tile([128, 128 + 2 * KH], BF16)
            nc.sync.dma_start(out=aux_sb, in_=aux_d.ap())
            big8_sb = acts.tile([128, KC, PCOLS], FP8)
            nc.scalar.dma_start(out=big8_sb, in_=big8_d.ap())

            eps_t = acts.tile([128, 1], F32)
            nc.vector.memset(eps_t, np.float32(EPS))
            warm_src = acts.tile([128, 256], BF16)
            nc.vector.memset(warm_src, np.float32(0.5))

            ident = aux_sb[:, 0:128]

            # ---- PE warm-up fillers: emitted first so they are the only
            # ready PE work at kernel start; keeps the array busy so the
            # HAM clock gate lifts (1.2 -> 2.4 GHz) before real matmuls ----
            for i in range(N_WARM):
                pw = ptrans.tile([128, 256], F32, tag="pt", name=f"pw_{i}")
                nc.tensor.matmul(pw, lhsT=warm_src[:, 0:128], rhs=warm_src,
                                 start=True, stop=True)

            with tc.high_priority():
                # ---- stats per row tile ----
                # aug_rhs[0,t] = mu[t]; aug_rhs[1,t] = sqrt(var[t]+eps)
                aug_rhs = acts.tile([2, ROWS], BF16)
                rstd_s = []
                stgs = []
                for r in range(RT):
                    xr = xr8_sb[:, r * C:(r + 1) * C]
                    bn6 = stats.tile([128, 6], F32, tag="bn6")
                    nc.vector.bn_stats(out=bn6, in_=xr)
                    mv = stats.tile([128, 2], F32, tag="mv")
                    nc.vector.bn_aggr(out=mv, in_=bn6)
                    stg = stats.tile([128, 2], BF16, tag="stg")
                    nc.scalar.activation(
                        out=stg[:, 1:2], in_=mv[:, 1:2],
                        func=mybir.ActivationFunctionType.Sqrt,
                        bias=eps_t, scale=1.0)
                    nc.vector.tensor_copy(out=stg[:, 0:1], in_=mv[:, 0:1])
                    stgs.append(stg)
                    # rstd/STOT for the final scale (off the aug path)
                    rstd = stats.tile([128, 1], F32, tag="rstd")
                    nc.vector.reciprocal(out=rstd, in_=stg[:, 1:2])
                    rs = stats.tile([128, 1], F32, tag="rs")
                    nc.vector.tensor_scalar_mul(rs, rstd, 1.0 / STOT)
                    rstd_s.append(rs)

                # ---- stat transposes, then augw assembly (PE idle window)
                for r in range(RT):
                    pt = ptrans.tile([2, 128], BF16, tag="pt", name=f"pt_{r}")
                    nc.tensor.transpose(pt, stgs[r], ident)
                    nc.vector.tensor_copy(
                        out=aug_rhs[:, r * 128:(r + 1) * 128], in_=pt)

                augw_sb = acts.tile([2, HH], BF16)
                for mc in range(KH):
                    pa = ptrans.tile([2, 128], BF16, tag="pt",
                                     name=f"pa_{mc}")
                    nc.tensor.transpose(
                        pa, aux_sb[:, 128 + 2 * mc:130 + 2 * mc], ident)
                    nc.vector.tensor_copy(
                        out=augw_sb[:, mc * 128:(mc + 1) * 128], in_=pa)

            # ---- raw mm1 (fp8 DR: both k-chunks in one matmul) ----
            ps1 = [pmm1.tile([128, ROWS], F32, tag=f"ps1_{i}", bufs=1,
                             name=f"ps1_{i}")
                   for i in range(KH)]
            for mc in range(KH):
                nc.tensor.matmul(
                    ps1[mc],
                    lhsT=big8_sb[:, :, PW1 + mc * 128:PW1 + (mc + 1) * 128],
                    rhs=big8_sb[:, :, 0:256],
                    start=True, stop=False,
                    perf_mode=DR,
                )

            # ---- aug matmul (bf16), then relu halves on both engines ----
            relu1 = acts.tile([128, KH, ROWS], FP8)
            for mc in range(KH):
                nc.tensor.matmul(
                    ps1[mc],
                    lhsT=augw_sb[:, mc * 128:(mc + 1) * 128],
                    rhs=aug_rhs,
                    start=False, stop=True,
                )
            for mc in range(KH):
                if mc % 2 == 0:
                    nc.scalar.activation(
                        out=relu1[:, mc, :], in_=ps1[mc],
                        func=mybir.ActivationFunctionType.Relu,
                        bias=0.0, scale=float(SR))
                else:
                    nc.vector.tensor_scalar(
                        out=relu1[:, mc, :], in0=ps1[mc],
                        scalar1=0.0, scalar2=float(SR),
                        op0=mybir.AluOpType.max,
                        op1=mybir.AluOpType.mult)

            # ---- mm2 (fp8 DR) + final rstd/STOT scale per row tile ----
            y_sb = acts.tile([128, RT, C], BF16)
            for r in range(RT):
                po = pmm2.tile([128, C], F32)
                for j in range(KH // 2):
                    nc.tensor.matmul(
                        po,
                        lhsT=relu1[:, 2 * j:2 * j + 2, r * 128:(r + 1) * 128],
                        rhs=big8_sb[:, :, PW2 + j * C:PW2 + (j + 1) * C],
                        start=(j == 0), stop=(j == KH // 2 - 1),
                        perf_mode=DR,
                    )
                if r == 0:
                    nc.vector.tensor_scalar_mul(y_sb[:, 0, :], po, rstd_s[0])
                    nc.sync.dma_start(out=y_d.ap()[:, :C], in_=y_sb[:, 0, :])
                else:
                    nc.scalar.activation(
                        out=y_sb[:, 1, :], in_=po,
                        func=mybir.ActivationFunctionType.Copy,
                        bias=0.0, scale=rstd_s[1])
                    nc.scalar.dma_start(out=y_d.ap()[:, C:],
                                        in_=y_sb[:, 1, :])

    nc.finalize()
    return nc


_NC_CACHE = None


def _get_nc():
    global _NC_CACHE
    if _NC_CACHE is None:
        _NC_CACHE = _build_nc()
    return _NC_CACHE


def _q8(a, scale):
    s = np.asarray(a, dtype=np.float64) * scale
    s = np.clip(s, -240.0, 240.0)
    return s.astype(np.float32).astype(FP8_NP)


def _pack_inputs(x, bp, g2, b2, W1, bf1, W2):
    x1 = (np.asarray(x, dtype=np.float64).reshape(B * T, C)
          + np.asarray(bp, dtype=np.float64))
    x1_f32 = x1.astype(np.float32)
    x1_f8 = x1_f32.astype(BF16_NP).astype(np.float32).astype(FP8_NP)

    w1t_eff = (np.asarray(W1).astype(np.float64).T
               * np.asarray(g2).astype(np.float64)[:, None])      # [C, HID]
    w1t_f8 = _q8(w1t_eff, S1)
    bf1_eff = (np.asarray(bf1).astype(np.float64)
               + np.asarray(b2).astype(np.float64)
               @ np.asarray(W1).astype(np.float64).T)             # [HID]
    # aug row0 = -sum_c of the scaled fp8 weights actually used
    s1_scaled = w1t_f8.astype(np.float64).sum(axis=0)             # S1-scaled
    aug0 = (-s1_scaled).astype(np.float32).astype(BF16_NP)
    aug1 = (bf1_eff * S1).astype(np.float32).astype(BF16_NP)
    w2t_f8 = _q8(np.asarray(W2, dtype=np.float64).T, S2)          # [HID, C]
    ident = np.eye(128, dtype=np.float32).astype(BF16_NP)

    in_maps = []
    for c in range(N_CORES):
        g, hf = c // 2, c % 2
        xg_f8 = x1_f8[g * ROWS:(g + 1) * ROWS]                    # [256, C]

        xr8 = np.empty((128, RT * C), dtype=FP8_NP)
        for r in range(RT):
            xr8[:, r * C:(r + 1) * C] = xg_f8[r * 128:(r + 1) * 128, :]

        aux = np.empty((128, 128 + 2 * KH), dtype=BF16_NP)
        aux[:, 0:128] = ident
        for mc in range(KH):
            sl = slice(hf * HH + mc * 128, hf * HH + (mc + 1) * 128)
            aux[:, 128 + 2 * mc] = aug0[sl]
            aux[:, 129 + 2 * mc] = aug1[sl]

        big8 = np.empty((128, KC, PCOLS), dtype=FP8_NP)
        w1h = w1t_f8[:, hf * HH:(hf + 1) * HH]                    # [C, HH]
        w2h = w2t_f8[hf * HH:(hf + 1) * HH]                       # [HH, C]
        for k in range(KC):
            big8[:, k, 0:PW1] = xg_f8[:, k * 128:(k + 1) * 128].T
            big8[:, k, PW1:PW2] = w1h[k * 128:(k + 1) * 128, :]
        for j in range(KH // 2):
            for q in range(2):
                mc = 2 * j + q
                big8[:, q, PW2 + j * C:PW2 + (j + 1) * C] = \
                    w2h[mc * 128:(mc + 1) * 128, :]

        in_maps.append({"xr8": xr8, "aux": aux, "big8": big8})
    return in_maps, x1_f32


def _make_in_maps(x, bp, g2, b2, W1, bf1, W2):
    in_maps, _ = _pack_inputs(x, bp, g2, b2, W1, bf1, W2)
    return in_maps


def kernel(x, Wt, Wp, bp, g1, b1, g2, b2, W1, bf1, W2, bf2):
    in_maps, x1_f32 = _pack_inputs(x, bp, g2, b2, W1, bf1, W2)
    nc = _get_nc()
    res = run_bass_kernel_spmd(nc, in_maps, list(range(N_CORES)))

    out = x1_f32.copy()                                       # residual x+bp
    for g in range(N_GROUPS):
        for hf in range(2):
            y = np.asarray(res.results[2 * g + hf]["y"]).astype(np.float32)
            for r in range(RT):
                out[g * ROWS + r * 128:g * ROWS + (r + 1) * 128, :] += \
                    y[:, r * C:(r + 1) * C]
    out = out + np.asarray(bf2, dtype=np.float32)
    return out.reshape(B, T, C).astype(np.float32)
